# revision 1
# baseline (speedup 1.0000x reference)
"""Locally-connected 2D conv (unshared weights), VALID, stride 2 — Trainium2 Bass kernel.

Problem (hardcoded):
  x:       (16, 32, 113, 113) f32
  weights: (56, 56, 32, 3, 3, 64) f32   (H_out, W_out, C_in, kh, kw, C_out)
  bias:    (56, 56, 64) f32
  out:     (16, 64, 56, 56) f32
  out[b,o,u,v] = sum_{c,q,r} x[b,c,2u+q,2v+r] * weights[u,v,c,q,r,o] + bias[u,v,o]

Sharding: H_out split across 8 cores (7 output rows each); each core reads only
its 1/8 of the 231MB weight tensor (the dominant traffic).

Host-side repack: weights/bias/x are rearranged with numpy into the exact SBUF
tile layouts so every device DMA moves >=20KB-contiguous runs per partition
(descriptor count is the DMA bottleneck on trn2: ~70ns/desc/engine), with
partition counts divisible by 16 so the HWDGE sprays descriptors across all 16
SDMA engines.

Per-core compute: for each output location (u,v):
  psum(o=64, b=16) += W_chunk[k, r*64+o].T @ X'[k, b]   for r in 0..2
with contraction k = (q,c) on 96 partitions. One PSUM accumulation group spans
a 28-v bank chunk (start on the first matmul, stop on the last; first write
per byte range overwrites, then accumulates). Bias is added by the DVE during
the PSUM->SBUF copy (broadcast over batch). Matmul operands are float32r
(single-pass PE, ~1.8e-4 rel err vs the f32 reference).
"""

import numpy as np

B = 16
C_IN = 32
C_OUT = 64
H_OUT = 56
W_OUT = 56
KK = 3
STRIDE = 2
H_IN = 113

N_CORES = 8
U_PER = H_OUT // N_CORES          # 7 output rows per core
ROWS_IN = (U_PER - 1) * STRIDE + KK  # 15 input rows per core
J_ROWS = ROWS_IN - (KK - 1)       # 13 rows stored per q-shifted copy
VCHUNK = 28                       # output cols per PSUM bank chunk
NCHUNK = W_OUT // VCHUNK          # 2 chunks per u
XFREE = B * J_ROWS * H_IN         # x' tile free size (f32 elems)
WFREE = VCHUNK * KK * C_OUT       # weight chunk free size (5376)
KPART = C_IN * KK                 # 96 contraction partitions (q,c)

_CACHE = {}


def _build():
    import concourse.mybir as mybir
    from concourse import bacc
    from concourse.tile import TileContext

    f32 = mybir.dt.float32
    f32r = mybir.dt.float32r
    nc = bacc.Bacc("TRN2", target_bir_lowering=False, debug=False,
                   num_devices=N_CORES)
    # Host-prepacked tensors (see _pack_core):
    #   xp[p, b*1469 + j*113 + w] = x[b, c, 2u0+q+j, w],  p = q*32+c
    #   wp[u, ch, p, v*192 + r*64 + o] = weights[u0+u, 28ch+v, c, q, r, o]
    #   bp[o, u*56 + v] = bias[u0+u, v, o]
    xp_in = nc.dram_tensor("xp", [KPART, XFREE], f32r,
                           kind="ExternalInput").ap()
    wp_in = nc.dram_tensor("wp", [U_PER, NCHUNK, KPART, WFREE], f32r,
                           kind="ExternalInput").ap()
    bp_in = nc.dram_tensor("bp", [C_OUT, U_PER * W_OUT], f32,
                           kind="ExternalInput").ap()
    y_out = nc.dram_tensor("y", [B, C_OUT, U_PER, W_OUT], f32,
                           kind="ExternalOutput").ap()

    with TileContext(nc) as tc:
        with tc.tile_pool(name="xpool", bufs=1) as xpool, \
             tc.tile_pool(name="wpool", bufs=3) as wpool, \
             tc.tile_pool(name="opool", bufs=1) as opool, \
             tc.tile_pool(name="pspool", bufs=4, space="PSUM") as pspool:

            # x/bias/out ride the ACT HWDGE ring so the weight stream on the
            # SP ring is never stuck behind them (FIFO per ring)
            xt = xpool.tile([KPART, XFREE], f32r)
            nc.scalar.dma_start(out=xt[:], in_=xp_in[:])
            xt3 = xt.rearrange("p (b hw) -> p b hw", b=B)

            bt = xpool.tile([C_OUT, U_PER * W_OUT], f32)
            nc.scalar.dma_start(out=bt[:], in_=bp_in[:])

            # output staging: partition o, free (b, u, v) -> contiguous dest runs
            out_all = opool.tile([C_OUT, B * U_PER * W_OUT], f32)
            oa3 = out_all.rearrange("p (b uv) -> p b uv", b=B)

            for u in range(U_PER):
                for ch in range(NCHUNK):
                    v0 = ch * VCHUNK
                    wt = wpool.tile([KPART, WFREE], f32r)
                    weng = nc.sync if (u * NCHUNK + ch) % 2 == 0 else nc.scalar
                    weng.dma_start(out=wt[:], in_=wp_in[u, ch])
                    wt3 = wt.rearrange("p (v ro) -> p v ro", v=VCHUNK)

                    ps = pspool.tile([C_OUT, VCHUNK * B], f32)
                    for vl in range(VCHUNK):
                        v = v0 + vl
                        for r in range(KK):
                            lhsT = wt3[:, vl:vl + 1,
                                       r * C_OUT:(r + 1) * C_OUT]
                            col = (2 * u) * H_IN + STRIDE * v + r
                            rhs = xt3[:, :, col:col + 1]
                            nc.tensor.matmul(
                                ps[:, vl * B:(vl + 1) * B], lhsT, rhs,
                                start=(vl == 0 and r == 0),
                                stop=(vl == VCHUNK - 1 and r == KK - 1),
                            )
                    ps3 = ps.rearrange("p (v b) -> p b v", v=VCHUNK)
                    uv = u * W_OUT + v0
                    bslice = bt[:, uv:uv + VCHUNK].unsqueeze(1).broadcast_to(
                        [C_OUT, B, VCHUNK])
                    nc.vector.tensor_add(
                        oa3[:, :, uv:uv + VCHUNK], ps3, bslice)

            ydst = y_out.rearrange("b o u v -> o b (u v)")
            nc.scalar.dma_start(out=ydst, in_=out_all.rearrange(
                "p (b uv) -> p b uv", b=B))

    nc.compile()
    return nc


def _get_nc():
    if "nc" not in _CACHE:
        _CACHE["nc"] = _build()
    return _CACHE["nc"]


def _pack_core(x, weights, bias, i):
    u0 = i * U_PER
    # x': (96, B*13*113); p = q*32+c holds x[b, c, 2u0+q+j, w]
    xs = x[:, :, STRIDE * u0:STRIDE * u0 + ROWS_IN, :]      # (B, C, 15, 113)
    xq = np.stack([xs[:, :, q:q + J_ROWS, :] for q in range(KK)], axis=0)
    xq = xq.transpose(0, 2, 1, 3, 4)                        # (q, c, b, j, w)
    xp = np.ascontiguousarray(xq.reshape(KPART, XFREE))

    # w': (U_PER, NCHUNK, 96, 5376); p = q*32+c, free (v, r, o)
    ws = weights[u0:u0 + U_PER].reshape(U_PER, NCHUNK, VCHUNK, C_IN, KK, KK,
                                        C_OUT)
    ws = ws.transpose(0, 1, 4, 3, 2, 5, 6)                  # (u, ch, q, c, v, r, o)
    wp = np.ascontiguousarray(ws.reshape(U_PER, NCHUNK, KPART, WFREE))

    # b': (64, 392): bp[o, u*56+v]
    bp = np.ascontiguousarray(
        bias[u0:u0 + U_PER].reshape(U_PER * W_OUT, C_OUT).T)
    return {"xp": xp, "wp": wp, "bp": bp}


def kernel(x, weights, bias, _trace=False, _tmpdir=None):
    from concourse.bass_utils import run_bass_kernel_spmd

    x = np.ascontiguousarray(x, dtype=np.float32)
    weights = np.ascontiguousarray(weights, dtype=np.float32)
    bias = np.ascontiguousarray(bias, dtype=np.float32)

    nc = _get_nc()
    core_ids = list(range(N_CORES))
    in_maps = [_pack_core(x, weights, bias, i) for i in core_ids]
    res = run_bass_kernel_spmd(nc, in_maps, core_ids, trace=_trace,
                               tmpdir=_tmpdir)
    out = np.concatenate([res.results[i]["y"] for i in core_ids], axis=2)
    if _trace:
        _CACHE["last_result"] = res
    return out



# revision 4
# speedup vs baseline: 2.5697x; 2.5697x over previous
"""Locally-connected 2D conv (unshared weights), VALID, stride 2 — Trainium2 Bass kernel.

Problem (hardcoded):
  x:       (16, 32, 113, 113) f32
  weights: (56, 56, 32, 3, 3, 64) f32   (H_out, W_out, C_in, kh, kw, C_out)
  bias:    (56, 56, 64) f32
  out:     (16, 64, 56, 56) f32
  out[b,o,u,v] = sum_{c,q,r} x[b,c,2u+q,2v+r] * weights[u,v,c,q,r,o] + bias[u,v,o]

Sharding: H_out split across 8 cores (7 output rows each); each core reads only
its 1/8 of the weight tensor (the dominant HBM traffic).

The kernel is DMA-bound (~174 GB/s/core payload ceiling, all 16 SDMA engines
saturated), so everything is about minimizing bytes:
  - weights and x are cast to fp16 on the host (tolerance 2e-2; fp16 keeps the
    error ~3e-4), halving the dominant stream,
  - x is packed as the 7 even row-slabs actually read (3 q-shifted copies on
    96 partitions), not whole rows (the f32 baseline DMAed 13 slabs of which
    6 were never read),
  - bias rides as a 97th contraction row (ones row in x's stationary tile,
    bias values in the weight stream at the r=1 tap) - no bias DMA, no
    broadcast add,
  - output is written back fp16 (|out| <= ~90, ulp 0.06) and cast on host.

DMA layout rules learned from traces: a DMA whose partition count is not a
multiple of 16 is NOT sprayed across the 16 SDMA engines (it lands on one
engine at ~26 GB/s), so every 97-row transfer is split into a 96-row DMA plus
a 1-row DMA. Weight tiles ride the SP HWDGE ring in consumption order;
x/outputs ride the ACT ring so a pending output DMA never FIFO-blocks a
weight prefetch.

Per-core compute: x is the PE *stationary* operand (LDWEIGHTS of 16 columns)
and the weights are the *moving* operand: for input column w, the taps
(v, r=w-2v) consume weight columns [v*192+r*64, +64) which are contiguous, so
an even w does one N=128 matmul for both taps. PSUM accumulates f32 in
one-bank chunks of 8 output columns ([16, 512] f32 = 2KB = the PSUM
zero-region granularity; start=True only arms pending-zero for the 2KB region
containing the first matmul's target, so a chunk must not span banks). The
DVE copies each finished chunk to fp16 SBUF staging; one DMA per u streams it
out.
"""

import numpy as np

B = 16
C_IN = 32
C_OUT = 64
H_OUT = 56
W_OUT = 56
KK = 3
STRIDE = 2
H_IN = 113

N_CORES = 8
U_PER = H_OUT // N_CORES          # 7 output rows per core
T_ROWS = U_PER                    # 7 even row-slabs per q-shifted copy
VCHUNK = 8                        # output cols per PSUM chunk (1 bank)
NCHUNK = W_OUT // VCHUNK          # 7 chunks per u
KPART = C_IN * KK + 1             # 96 contraction partitions (q,c) + bias row
XFREE = T_ROWS * H_IN * B         # x' tile free size (12656 fp16 elems)
WFREE = W_OUT * KK * C_OUT        # weight tile free size per u (10752)
RO = KK * C_OUT                   # 192: cols per v in the weight stream

_CACHE = {}


def _chunk_matmuls(ch):
    """Matmuls for one 8-v psum chunk: (w, psum_off_f32, col_off, ncols).

    psum offsets are f32 elements relative to the [16, 512] chunk tile; col
    offsets are relative to the per-u weight tile. The two taps of an even w
    fuse into one N=128 matmul when both fall in this chunk.

    PSUM has_written is per-byte and each matmul must be uniformly
    first-write or accumulate, so the odd-w (r=1) matmuls run first - each is
    the unique first writer of its 64-col v region - and every even-w matmul
    (fused or not) then purely accumulates.
    """
    v0 = ch * VCHUNK
    odd, even = [], []
    for w in range(2 * v0, 2 * (v0 + VCHUNK - 1) + 3):
        pairs = []
        for r in (2, 1, 0):
            v = (w - r) // 2
            if 2 * v + r == w and v0 <= v < v0 + VCHUNK and 0 <= v < W_OUT:
                pairs.append((v, r))
        if not pairs:
            continue
        if len(pairs) == 2:
            v, r = pairs[0]
            even.append((w, (v - v0) * C_OUT, v * RO + r * C_OUT, 2 * C_OUT))
        else:
            for v, r in pairs:
                dst = odd if w % 2 else even
                dst.append((w, (v - v0) * C_OUT, v * RO + r * C_OUT, C_OUT))
    return odd + even


def _build():
    import concourse.mybir as mybir
    from concourse import bacc
    from concourse.tile import TileContext

    f16 = mybir.dt.float16
    nc = bacc.Bacc("TRN2", target_bir_lowering=False, debug=False,
                   num_devices=N_CORES)
    # Host-prepacked tensors (see _pack_core):
    #   xp[p, t*113*16 + w*16 + b] = x[b, c, 2u0+q+2t, w],  p = q*32+c; row 96 = 1.0
    #   wp[u, p, v*192 + r*64 + o] = weights[u0+u, v, c, q, r, o];
    #     row 96 = bias[u0+u, v, o] at r==1, else 0
    #   y[u, b, v*64 + o] fp16
    xp_in = nc.dram_tensor("xp", [KPART, XFREE], f16,
                           kind="ExternalInput").ap()
    wp_in = nc.dram_tensor("wp", [U_PER, KPART, WFREE], f16,
                           kind="ExternalInput").ap()
    y_out = nc.dram_tensor("y", [U_PER, B, W_OUT * C_OUT], f16,
                           kind="ExternalOutput").ap()

    with TileContext(nc) as tc:
        with tc.tile_pool(name="xpool", bufs=1) as xpool, \
             tc.tile_pool(name="wpool", bufs=3) as wpool, \
             tc.tile_pool(name="opool", bufs=2) as opool, \
             tc.tile_pool(name="pspool", bufs=4, space="PSUM") as pspool:

            # x + outputs on the ACT ring; weights on the SP ring.
            # Partition counts on every dma_start are multiples of 16 (plus a
            # 1-row fixup) so the HWDGE sprays descriptors across all 16 SDMA
            # engines.
            xt = xpool.tile([KPART, XFREE], f16)
            nc.scalar.dma_start(out=xt[0:96, :], in_=xp_in[0:96, :])
            nc.scalar.dma_start(out=xt[96:97, :], in_=xp_in[96:97, :])
            # (p, t*113+w, b) view for the stationary slices
            xt3 = xt.rearrange("p (tw b) -> p tw b", b=B)

            for u in range(U_PER):
                wt = wpool.tile([KPART, WFREE], f16)
                nc.sync.dma_start(out=wt[0:96, :], in_=wp_in[u, 0:96])
                nc.sync.dma_start(out=wt[96:97, :], in_=wp_in[u, 96:97])

                stage = opool.tile([B, W_OUT * C_OUT], f16)
                for ch in range(NCHUNK):
                    ps = pspool.tile([B, VCHUNK * C_OUT], mybir.dt.float32)
                    mms = _chunk_matmuls(ch)
                    for i, (w, ps_off, col, ncol) in enumerate(mms):
                        lhsT = xt3[:, u * H_IN + w:u * H_IN + w + 1, :]
                        nc.tensor.matmul(
                            ps[:, ps_off:ps_off + ncol],
                            lhsT, wt[:, col:col + ncol],
                            start=(i == 0), stop=(i == len(mms) - 1),
                        )
                    nc.vector.tensor_copy(
                        out=stage[:, ch * VCHUNK * C_OUT:
                                  (ch + 1) * VCHUNK * C_OUT],
                        in_=ps[:])
                nc.scalar.dma_start(out=y_out[u], in_=stage[:])

    nc.compile()
    return nc


def _get_nc():
    if "nc" not in _CACHE:
        _CACHE["nc"] = _build()
    return _CACHE["nc"]


def _pack_core(x16, w16, b16, i):
    u0 = i * U_PER
    # x': (97, 7*113*16); p = q*32+c holds x[b, c, 2u0+q+2t, w] at (t, w, b)
    xs = x16[:, :, STRIDE * u0:STRIDE * u0 + 2 * U_PER + 1, :]  # (B,C,15,113)
    xq = np.stack([xs[:, :, q:q + 2 * U_PER - 1:STRIDE, :] for q in range(KK)],
                  axis=0)                                   # (q, b, c, t, w)
    xq = xq.transpose(0, 2, 3, 4, 1)                        # (q, c, t, w, b)
    xp = np.empty((KPART, XFREE), dtype=np.float16)
    xp[:KPART - 1] = xq.reshape(KPART - 1, XFREE)
    xp[KPART - 1] = np.float16(1.0)

    # w': (7, 97, 10752); p = q*32+c, free (v, r, o); row 96 = bias at r==1
    ws = w16[u0:u0 + U_PER]                             # (u, v, c, q, r, o)
    ws = ws.transpose(0, 3, 2, 1, 4, 5)                 # (u, q, c, v, r, o)
    wp = np.empty((U_PER, KPART, WFREE), dtype=np.float16)
    wp[:, :KPART - 1] = ws.reshape(U_PER, KPART - 1, WFREE)
    brow = np.zeros((U_PER, W_OUT, KK, C_OUT), dtype=np.float16)
    brow[:, :, 1, :] = b16[u0:u0 + U_PER]
    wp[:, KPART - 1] = brow.reshape(U_PER, WFREE)
    return {"xp": np.ascontiguousarray(xp), "wp": np.ascontiguousarray(wp)}


def kernel(x, weights, bias, _trace=False, _tmpdir=None):
    from concourse.bass_utils import run_bass_kernel_spmd

    x16 = np.asarray(x, dtype=np.float16)
    w16 = np.asarray(weights, dtype=np.float16)
    b16 = np.asarray(bias, dtype=np.float16)

    nc = _get_nc()
    core_ids = list(range(N_CORES))
    in_maps = [_pack_core(x16, w16, b16, i) for i in core_ids]
    res = run_bass_kernel_spmd(nc, in_maps, core_ids, trace=_trace,
                               tmpdir=_tmpdir)
    # y[u, b, v*64+o] per core -> out[b, o, u0+u, v]
    outs = []
    for i in core_ids:
        y = res.results[i]["y"].reshape(U_PER, B, W_OUT, C_OUT)
        outs.append(y.transpose(1, 3, 0, 2))
    out = np.concatenate(outs, axis=2).astype(np.float32)
    if _trace:
        _CACHE["last_result"] = res
    return out


# revision 6
# speedup vs baseline: 2.6034x; 1.0131x over previous
"""Locally-connected 2D conv (unshared weights), VALID, stride 2 — Trainium2 Bass kernel.

Problem (hardcoded):
  x:       (16, 32, 113, 113) f32
  weights: (56, 56, 32, 3, 3, 64) f32   (H_out, W_out, C_in, kh, kw, C_out)
  bias:    (56, 56, 64) f32
  out:     (16, 64, 56, 56) f32
  out[b,o,u,v] = sum_{c,q,r} x[b,c,2u+q,2v+r] * weights[u,v,c,q,r,o] + bias[u,v,o]

Sharding: H_out split across 8 cores (7 output rows each); each core reads only
its 1/8 of the weight tensor (the dominant HBM traffic).

The kernel is DMA-bound (~174 GB/s/core payload ceiling, all 16 SDMA engines
saturated), so everything is about minimizing bytes:
  - weights and x are cast to fp16 on the host (tolerance 2e-2; fp16 keeps the
    error ~3e-4), halving the dominant stream,
  - x is packed as the 7 even row-slabs actually read (3 q-shifted copies on
    96 partitions), not whole rows (the f32 baseline DMAed 13 slabs of which
    6 were never read),
  - bias rides as a 97th contraction row (ones row in x's stationary tile,
    bias values in the weight stream at the r=1 tap) - no bias DMA, no
    broadcast add,
  - output is written back fp16 (|out| <= ~90, ulp 0.06) and cast on host.

DMA layout rules learned from traces: a DMA whose partition count is not a
multiple of 16 is NOT sprayed across the 16 SDMA engines (it lands on one
engine at ~26 GB/s), so every 97-row transfer is split into a 96-row DMA plus
a 1-row DMA. Weight tiles ride the SP HWDGE ring in consumption order;
x/outputs ride the ACT ring so a pending output DMA never FIFO-blocks a
weight prefetch.

Per-core compute: x is the PE *stationary* operand (LDWEIGHTS of 16 columns)
and the weights are the *moving* operand: for input column w, the taps
(v, r=w-2v) consume weight columns [v*192+r*64, +64) which are contiguous, so
an even w does one N=128 matmul for both taps. PSUM accumulates f32 in
one-bank chunks of 8 output columns ([16, 512] f32 = 2KB = the PSUM
zero-region granularity; start=True only arms pending-zero for the 2KB region
containing the first matmul's target, so a chunk must not span banks). The
DVE copies each finished chunk to fp16 SBUF staging; one DMA per u streams it
out.
"""

import numpy as np

B = 16
C_IN = 32
C_OUT = 64
H_OUT = 56
W_OUT = 56
KK = 3
STRIDE = 2
H_IN = 113

N_CORES = 8
U_PER = H_OUT // N_CORES          # 7 output rows per core
T_ROWS = U_PER                    # 7 even row-slabs per q-shifted copy
VCHUNK = 8                        # output cols per PSUM chunk (1 bank)
NCHUNK = W_OUT // VCHUNK          # 7 chunks per u
KPART = C_IN * KK + 1             # 96 contraction partitions (q,c) + bias row
XFREE = T_ROWS * H_IN * B         # x' tile free size (12656 fp16 elems)
WFREE = W_OUT * KK * C_OUT        # weight tile free size per u (10752)
RO = KK * C_OUT                   # 192: cols per v in the weight stream

_CACHE = {}


def _chunk_matmuls(ch):
    """Matmuls for one 8-v psum chunk: (w, psum_off_f32, col_off, ncols).

    psum offsets are f32 elements relative to the [16, 512] chunk tile; col
    offsets are relative to the per-u weight tile. The two taps of an even w
    fuse into one N=128 matmul when both fall in this chunk.

    PSUM has_written is per-byte and each matmul must be uniformly
    first-write or accumulate, so the odd-w (r=1) matmuls run first - each is
    the unique first writer of its 64-col v region - and every even-w matmul
    (fused or not) then purely accumulates.
    """
    v0 = ch * VCHUNK
    odd, even = [], []
    for w in range(2 * v0, 2 * (v0 + VCHUNK - 1) + 3):
        pairs = []
        for r in (2, 1, 0):
            v = (w - r) // 2
            if 2 * v + r == w and v0 <= v < v0 + VCHUNK and 0 <= v < W_OUT:
                pairs.append((v, r))
        if not pairs:
            continue
        if len(pairs) == 2:
            v, r = pairs[0]
            even.append((w, (v - v0) * C_OUT, v * RO + r * C_OUT, 2 * C_OUT))
        else:
            for v, r in pairs:
                dst = odd if w % 2 else even
                dst.append((w, (v - v0) * C_OUT, v * RO + r * C_OUT, C_OUT))
    return odd + even


def _build():
    import concourse.mybir as mybir
    from concourse import bacc
    from concourse.tile import TileContext

    f16 = mybir.dt.float16
    nc = bacc.Bacc("TRN2", target_bir_lowering=False, debug=False,
                   num_devices=N_CORES)
    # Host-prepacked tensors (see _pack_core):
    #   xp[p, t*113*16 + w*16 + b] = x[b, c, 2u0+q+2t, w],  p = q*32+c; row 96 = 1.0
    #   wp[u, p, v*192 + r*64 + o] = weights[u0+u, v, c, q, r, o];
    #     row 96 = bias[u0+u, v, o] at r==1, else 0
    #   y[u, b, v*64 + o] fp16
    xp_in = nc.dram_tensor("xp", [KPART, XFREE], f16,
                           kind="ExternalInput").ap()
    wp_in = nc.dram_tensor("wp", [U_PER, KPART, WFREE], f16,
                           kind="ExternalInput").ap()
    y_out = nc.dram_tensor("y", [U_PER, B, W_OUT * C_OUT], f16,
                           kind="ExternalOutput").ap()

    with TileContext(nc) as tc:
        with tc.tile_pool(name="xpool", bufs=1) as xpool, \
             tc.tile_pool(name="wpool", bufs=3) as wpool, \
             tc.tile_pool(name="w6pool", bufs=NCHUNK) as w6pool, \
             tc.tile_pool(name="opool", bufs=2) as opool, \
             tc.tile_pool(name="pspool", bufs=4, space="PSUM") as pspool:

            # x + outputs on the ACT ring; weights on the SP ring.
            # Partition counts on every dma_start are multiples of 16 (plus a
            # 1-row fixup) so the HWDGE sprays descriptors across all 16 SDMA
            # engines.
            xt = xpool.tile([KPART, XFREE], f16)
            nc.scalar.dma_start(out=xt[0:96, :], in_=xp_in[0:96, :])
            nc.scalar.dma_start(out=xt[96:97, :], in_=xp_in[96:97, :])
            # (p, t*113+w, b) view for the stationary slices
            xt3 = xt.rearrange("p (tw b) -> p tw b", b=B)

            # wp viewed per 8-v chunk so the last u can stream at chunk
            # granularity (tapers the pipeline-drain tail)
            wp4 = wp_in.rearrange("u p (ch f) -> u ch p f", ch=NCHUNK)
            CHF = WFREE // NCHUNK                     # 1536 cols per chunk
            for u in range(U_PER):
                last = u == U_PER - 1
                if not last:
                    wt = wpool.tile([KPART, WFREE], f16)
                    nc.sync.dma_start(out=wt[0:96, :], in_=wp_in[u, 0:96])
                    nc.sync.dma_start(out=wt[96:97, :], in_=wp_in[u, 96:97])

                stage = opool.tile([B, W_OUT * C_OUT], f16)
                for ch in range(NCHUNK):
                    if last:
                        wtc = w6pool.tile([KPART, CHF], f16)
                        nc.sync.dma_start(out=wtc[0:96, :],
                                          in_=wp4[u, ch, 0:96])
                        nc.sync.dma_start(out=wtc[96:97, :],
                                          in_=wp4[u, ch, 96:97])
                    ps = pspool.tile([B, VCHUNK * C_OUT], mybir.dt.float32)
                    mms = _chunk_matmuls(ch)
                    for i, (w, ps_off, col, ncol) in enumerate(mms):
                        lhsT = xt3[:, u * H_IN + w:u * H_IN + w + 1, :]
                        src = (wtc[:, col - ch * CHF:col - ch * CHF + ncol]
                               if last else wt[:, col:col + ncol])
                        nc.tensor.matmul(
                            ps[:, ps_off:ps_off + ncol],
                            lhsT, src,
                            start=(i == 0), stop=(i == len(mms) - 1),
                        )
                    nc.vector.tensor_copy(
                        out=stage[:, ch * VCHUNK * C_OUT:
                                  (ch + 1) * VCHUNK * C_OUT],
                        in_=ps[:])
                nc.scalar.dma_start(out=y_out[u], in_=stage[:])

    nc.compile()
    return nc


def _get_nc():
    if "nc" not in _CACHE:
        _CACHE["nc"] = _build()
    return _CACHE["nc"]


def _pack_core(x16, w16, b16, i):
    u0 = i * U_PER
    # x': (97, 7*113*16); p = q*32+c holds x[b, c, 2u0+q+2t, w] at (t, w, b)
    xs = x16[:, :, STRIDE * u0:STRIDE * u0 + 2 * U_PER + 1, :]  # (B,C,15,113)
    xq = np.stack([xs[:, :, q:q + 2 * U_PER - 1:STRIDE, :] for q in range(KK)],
                  axis=0)                                   # (q, b, c, t, w)
    xq = xq.transpose(0, 2, 3, 4, 1)                        # (q, c, t, w, b)
    xp = np.empty((KPART, XFREE), dtype=np.float16)
    xp[:KPART - 1] = xq.reshape(KPART - 1, XFREE)
    xp[KPART - 1] = np.float16(1.0)

    # w': (7, 97, 10752); p = q*32+c, free (v, r, o); row 96 = bias at r==1
    ws = w16[u0:u0 + U_PER]                             # (u, v, c, q, r, o)
    ws = ws.transpose(0, 3, 2, 1, 4, 5)                 # (u, q, c, v, r, o)
    wp = np.empty((U_PER, KPART, WFREE), dtype=np.float16)
    wp[:, :KPART - 1] = ws.reshape(U_PER, KPART - 1, WFREE)
    brow = np.zeros((U_PER, W_OUT, KK, C_OUT), dtype=np.float16)
    brow[:, :, 1, :] = b16[u0:u0 + U_PER]
    wp[:, KPART - 1] = brow.reshape(U_PER, WFREE)
    return {"xp": np.ascontiguousarray(xp), "wp": np.ascontiguousarray(wp)}


def kernel(x, weights, bias, _trace=False, _tmpdir=None):
    from concourse.bass_utils import run_bass_kernel_spmd

    x16 = np.asarray(x, dtype=np.float16)
    w16 = np.asarray(weights, dtype=np.float16)
    b16 = np.asarray(bias, dtype=np.float16)

    nc = _get_nc()
    core_ids = list(range(N_CORES))
    in_maps = [_pack_core(x16, w16, b16, i) for i in core_ids]
    res = run_bass_kernel_spmd(nc, in_maps, core_ids, trace=_trace,
                               tmpdir=_tmpdir)
    # y[u, b, v*64+o] per core -> out[b, o, u0+u, v]
    outs = []
    for i in core_ids:
        y = res.results[i]["y"].reshape(U_PER, B, W_OUT, C_OUT)
        outs.append(y.transpose(1, 3, 0, 2))
    out = np.concatenate(outs, axis=2).astype(np.float32)
    if _trace:
        _CACHE["last_result"] = res
    return out


# revision 7
# speedup vs baseline: 2.6294x; 1.0100x over previous
"""Locally-connected 2D conv (unshared weights), VALID, stride 2 — Trainium2 Bass kernel.

Problem (hardcoded):
  x:       (16, 32, 113, 113) f32
  weights: (56, 56, 32, 3, 3, 64) f32   (H_out, W_out, C_in, kh, kw, C_out)
  bias:    (56, 56, 64) f32
  out:     (16, 64, 56, 56) f32
  out[b,o,u,v] = sum_{c,q,r} x[b,c,2u+q,2v+r] * weights[u,v,c,q,r,o] + bias[u,v,o]

Sharding: H_out split across 8 cores (7 output rows each); each core reads only
its 1/8 of the weight tensor (the dominant HBM traffic).

The kernel is DMA-bound (~174 GB/s/core payload ceiling, all 16 SDMA engines
saturated), so everything is about minimizing bytes:
  - weights and x are cast to fp16 on the host (tolerance 2e-2; fp16 keeps the
    error ~3e-4), halving the dominant stream,
  - x is packed as the 7 even row-slabs actually read (3 q-shifted copies on
    96 partitions), not whole rows (the f32 baseline DMAed 13 slabs of which
    6 were never read),
  - bias rides as a 97th contraction row (ones row in x's stationary tile,
    bias values in the weight stream at the r=1 tap) - no bias DMA, no
    broadcast add,
  - output is written back fp16 (|out| <= ~90, ulp 0.06) and cast on host.

DMA layout rules learned from traces: a DMA whose partition count is not a
multiple of 16 is NOT sprayed across the 16 SDMA engines (it lands on one
engine at ~26 GB/s), so every 97-row transfer is split into a 96-row DMA plus
a 1-row DMA. Weight tiles ride the SP HWDGE ring in consumption order;
x/outputs ride the ACT ring so a pending output DMA never FIFO-blocks a
weight prefetch.

Per-core compute: x is the PE *stationary* operand (LDWEIGHTS of 16 columns)
and the weights are the *moving* operand: for input column w, the taps
(v, r=w-2v) consume weight columns [v*192+r*64, +64) which are contiguous, so
an even w does one N=128 matmul for both taps. PSUM accumulates f32 in
one-bank chunks of 8 output columns ([16, 512] f32 = 2KB = the PSUM
zero-region granularity; start=True only arms pending-zero for the 2KB region
containing the first matmul's target, so a chunk must not span banks). The
DVE copies each finished chunk to fp16 SBUF staging; one DMA per u streams it
out.
"""

import numpy as np

B = 16
C_IN = 32
C_OUT = 64
H_OUT = 56
W_OUT = 56
KK = 3
STRIDE = 2
H_IN = 113

N_CORES = 8
U_PER = H_OUT // N_CORES          # 7 output rows per core
T_ROWS = U_PER                    # 7 even row-slabs per q-shifted copy
VCHUNK = 8                        # output cols per PSUM chunk (1 bank)
NCHUNK = W_OUT // VCHUNK          # 7 chunks per u
KPART = C_IN * KK + 1             # 96 contraction partitions (q,c) + bias row
XFREE = T_ROWS * H_IN * B         # x' tile free size (12656 fp16 elems)
WFREE = W_OUT * KK * C_OUT        # weight tile free size per u (10752)
RO = KK * C_OUT                   # 192: cols per v in the weight stream

_CACHE = {}


def _chunk_matmuls(ch):
    """Matmuls for one 8-v psum chunk: (w, psum_off_f32, col_off, ncols).

    psum offsets are f32 elements relative to the [16, 512] chunk tile; col
    offsets are relative to the per-u weight tile. The two taps of an even w
    fuse into one N=128 matmul when both fall in this chunk.

    PSUM has_written is per-byte and each matmul must be uniformly
    first-write or accumulate, so the odd-w (r=1) matmuls run first - each is
    the unique first writer of its 64-col v region - and every even-w matmul
    (fused or not) then purely accumulates.
    """
    v0 = ch * VCHUNK
    odd, even = [], []
    for w in range(2 * v0, 2 * (v0 + VCHUNK - 1) + 3):
        pairs = []
        for r in (2, 1, 0):
            v = (w - r) // 2
            if 2 * v + r == w and v0 <= v < v0 + VCHUNK and 0 <= v < W_OUT:
                pairs.append((v, r))
        if not pairs:
            continue
        if len(pairs) == 2:
            v, r = pairs[0]
            even.append((w, (v - v0) * C_OUT, v * RO + r * C_OUT, 2 * C_OUT))
        else:
            for v, r in pairs:
                dst = odd if w % 2 else even
                dst.append((w, (v - v0) * C_OUT, v * RO + r * C_OUT, C_OUT))
    return odd + even


def _build():
    import concourse.mybir as mybir
    from concourse import bacc
    from concourse.tile import TileContext

    f16 = mybir.dt.float16
    nc = bacc.Bacc("TRN2", target_bir_lowering=False, debug=False,
                   num_devices=N_CORES)
    # Host-prepacked tensors (see _pack_core):
    #   xp[p, t*113*16 + w*16 + b] = x[b, c, 2u0+q+2t, w],  p = q*32+c; row 96 = 1.0
    #   wp[u, p, v*192 + r*64 + o] = weights[u0+u, v, c, q, r, o];
    #     row 96 = bias[u0+u, v, o] at r==1, else 0
    #   y[u, b, v*64 + o] fp16
    xp_in = nc.dram_tensor("xp", [KPART, XFREE], f16,
                           kind="ExternalInput").ap()
    wp_in = nc.dram_tensor("wp", [U_PER, KPART, WFREE], f16,
                           kind="ExternalInput").ap()
    y_out = nc.dram_tensor("y", [U_PER, B, W_OUT * C_OUT], f16,
                           kind="ExternalOutput").ap()

    with TileContext(nc) as tc:
        with tc.tile_pool(name="xpool", bufs=1) as xpool, \
             tc.tile_pool(name="wpool", bufs=4) as wpool, \
             tc.tile_pool(name="w6pool", bufs=NCHUNK) as w6pool, \
             tc.tile_pool(name="opool", bufs=4) as opool, \
             tc.tile_pool(name="pspool", bufs=8, space="PSUM") as pspool:

            # x + outputs on the ACT ring; weights on the SP ring.
            # Partition counts on every dma_start are multiples of 16 (plus a
            # 1-row fixup) so the HWDGE sprays descriptors across all 16 SDMA
            # engines.
            xt = xpool.tile([KPART, XFREE], f16)
            nc.scalar.dma_start(out=xt[0:96, :], in_=xp_in[0:96, :])
            nc.scalar.dma_start(out=xt[96:97, :], in_=xp_in[96:97, :])
            # (p, t*113+w, b) view for the stationary slices
            xt3 = xt.rearrange("p (tw b) -> p tw b", b=B)

            # wp viewed per 8-v chunk so the last u can stream at chunk
            # granularity (tapers the pipeline-drain tail)
            wp4 = wp_in.rearrange("u p (ch f) -> u ch p f", ch=NCHUNK)
            CHF = WFREE // NCHUNK                     # 1536 cols per chunk
            for u in range(U_PER):
                last = u == U_PER - 1
                if not last:
                    wt = wpool.tile([KPART, WFREE], f16)
                    nc.sync.dma_start(out=wt[0:96, :], in_=wp_in[u, 0:96])
                    nc.sync.dma_start(out=wt[96:97, :], in_=wp_in[u, 96:97])

                stage = opool.tile([B, W_OUT * C_OUT], f16)
                for ch in range(NCHUNK):
                    if last:
                        wtc = w6pool.tile([KPART, CHF], f16)
                        nc.sync.dma_start(out=wtc[0:96, :],
                                          in_=wp4[u, ch, 0:96])
                        nc.sync.dma_start(out=wtc[96:97, :],
                                          in_=wp4[u, ch, 96:97])
                    ps = pspool.tile([B, VCHUNK * C_OUT], mybir.dt.float32)
                    mms = _chunk_matmuls(ch)
                    for i, (w, ps_off, col, ncol) in enumerate(mms):
                        lhsT = xt3[:, u * H_IN + w:u * H_IN + w + 1, :]
                        src = (wtc[:, col - ch * CHF:col - ch * CHF + ncol]
                               if last else wt[:, col:col + ncol])
                        nc.tensor.matmul(
                            ps[:, ps_off:ps_off + ncol],
                            lhsT, src,
                            start=(i == 0), stop=(i == len(mms) - 1),
                        )
                    nc.vector.tensor_copy(
                        out=stage[:, ch * VCHUNK * C_OUT:
                                  (ch + 1) * VCHUNK * C_OUT],
                        in_=ps[:])
                nc.scalar.dma_start(out=y_out[u], in_=stage[:])

    nc.compile()
    return nc


def _get_nc():
    if "nc" not in _CACHE:
        _CACHE["nc"] = _build()
    return _CACHE["nc"]


def _pack_core(x16, w16, b16, i):
    u0 = i * U_PER
    # x': (97, 7*113*16); p = q*32+c holds x[b, c, 2u0+q+2t, w] at (t, w, b)
    xs = x16[:, :, STRIDE * u0:STRIDE * u0 + 2 * U_PER + 1, :]  # (B,C,15,113)
    xq = np.stack([xs[:, :, q:q + 2 * U_PER - 1:STRIDE, :] for q in range(KK)],
                  axis=0)                                   # (q, b, c, t, w)
    xq = xq.transpose(0, 2, 3, 4, 1)                        # (q, c, t, w, b)
    xp = np.empty((KPART, XFREE), dtype=np.float16)
    xp[:KPART - 1] = xq.reshape(KPART - 1, XFREE)
    xp[KPART - 1] = np.float16(1.0)

    # w': (7, 97, 10752); p = q*32+c, free (v, r, o); row 96 = bias at r==1
    ws = w16[u0:u0 + U_PER]                             # (u, v, c, q, r, o)
    ws = ws.transpose(0, 3, 2, 1, 4, 5)                 # (u, q, c, v, r, o)
    wp = np.empty((U_PER, KPART, WFREE), dtype=np.float16)
    wp[:, :KPART - 1] = ws.reshape(U_PER, KPART - 1, WFREE)
    brow = np.zeros((U_PER, W_OUT, KK, C_OUT), dtype=np.float16)
    brow[:, :, 1, :] = b16[u0:u0 + U_PER]
    wp[:, KPART - 1] = brow.reshape(U_PER, WFREE)
    return {"xp": np.ascontiguousarray(xp), "wp": np.ascontiguousarray(wp)}


def kernel(x, weights, bias, _trace=False, _tmpdir=None):
    from concourse.bass_utils import run_bass_kernel_spmd

    x16 = np.asarray(x, dtype=np.float16)
    w16 = np.asarray(weights, dtype=np.float16)
    b16 = np.asarray(bias, dtype=np.float16)

    nc = _get_nc()
    core_ids = list(range(N_CORES))
    in_maps = [_pack_core(x16, w16, b16, i) for i in core_ids]
    res = run_bass_kernel_spmd(nc, in_maps, core_ids, trace=_trace,
                               tmpdir=_tmpdir)
    # y[u, b, v*64+o] per core -> out[b, o, u0+u, v]
    outs = []
    for i in core_ids:
        y = res.results[i]["y"].reshape(U_PER, B, W_OUT, C_OUT)
        outs.append(y.transpose(1, 3, 0, 2))
    out = np.concatenate(outs, axis=2).astype(np.float32)
    if _trace:
        _CACHE["last_result"] = res
    return out
